# revision 27
# baseline (speedup 1.0000x reference)
"""Trainium2 Bass kernel for a local-attention transformer block (v2).

Per batch element (one NeuronCore each, 8 cores):
  y   = LN(x)  (gamma/beta and AdaLN scale/shift folded into qkv weights host-side)
  q,k,v = y @ qkv_w' + b'   (k bias dropped: softmax-invariant; v bias folded into proj bias)
  attn: each 128-token window attends [prev|cur|next]; keys-stationary sims,
        ones-column-in-v denominator trick, DVE-fused normalize from PSUM.
  x1  = x + attn @ proj_w + pb'   (pb' = proj_b + v_bias @ proj_w, per batch)
  out = x1 + gelu(LN2(x1) @ w1' + b1') @ w2 + b2   (ln2 g/b folded into w1/b1)

6-deep software pipeline per 512-token group (iteration i):
  prep(i) | qkv(i-1) | attn+proj(i-2) | ln2+h2T(i-3) | (h2T transit) | mlp(i-5)
PE order per iteration: MLP1, [slots: QK/V + sims + AV], MLP2, proj —
keeps the PE continuously busy (p-state) while ACT grinds exps/gelus.
ACT does exactly 3 table loads per iteration (gelu, exp, abs_rsqrt).
"""

import numpy as np
from contextlib import ExitStack

import concourse.bass as bass
import concourse.tile as tile
from concourse import bacc, mybir
from concourse import bass_utils

F32 = mybir.dt.float32
F16 = mybir.dt.float16
AF = mybir.ActivationFunctionType
AL = mybir.AluOpType

DIM = 512
HEADS = 8
HD = 64
FF = 2048
WIN = 128
NTOK = 8192
EPS = 1e-5
GRP = 512
TS = 128


def _bcast_row(dram_ap, offset, n):
    return bass.AP(tensor=dram_ap.tensor, offset=offset, ap=[[0, 128], [1, n]])


def _col_view(dram_ap, offset, ncol):
    return bass.AP(tensor=dram_ap.tensor, offset=offset, ap=[[1, 128], [128, ncol]])


def _sim_spans(g, n_blocks):
    """[(j, qlo, qhi)] for group g: key block j vs in-group query columns."""
    out = []
    for j in range(4 * g - 1, 4 * g + 5):
        if 0 <= j < n_blocks:
            qlo = (max(j - 1, 4 * g) - 4 * g) * WIN
            qhi = (min(j + 1, 4 * g + 3, n_blocks - 1) - 4 * g) * WIN + WIN
            out.append((j, qlo, qhi))
    return out


def _av_plan(g, n_blocks):
    """[(j, a, b, start)] accumulation plan into P_av[65, 512]."""
    plan = [(4 * g, 0, 256, True), (4 * g + 1, 0, 256, False)]
    if 4 * g - 1 >= 0:
        plan.append((4 * g - 1, 0, 128, False))
    plan.append((4 * g + 2, 128, 256, False))
    plan += [(4 * g + 2, 256, 512, True), (4 * g + 3, 256, 512, False),
             (4 * g + 1, 256, 384, False)]
    if 4 * g + 4 < n_blocks:
        plan.append((4 * g + 4, 384, 512, False))
    return plan


def build(n_tok=NTOK):
    n_groups = n_tok // GRP
    n_blocks = n_tok // WIN
    nc = bacc.Bacc("TRN2", target_bir_lowering=False, debug=False)

    x_d = nc.dram_tensor("x", [n_tok, DIM], F32, kind="ExternalInput")
    qkvw_d = nc.dram_tensor("qkvw", [DIM, 3 * DIM], F16, kind="ExternalInput")
    qb_d = nc.dram_tensor("qb", [DIM], F32, kind="ExternalInput")
    projw_d = nc.dram_tensor("projw", [DIM, DIM], F16, kind="ExternalInput")
    projb_d = nc.dram_tensor("projb", [DIM], F32, kind="ExternalInput")
    w1_d = nc.dram_tensor("w1", [DIM, FF], F16, kind="ExternalInput")
    b1_d = nc.dram_tensor("b1", [FF], F32, kind="ExternalInput")
    w2_d = nc.dram_tensor("w2", [FF, DIM], F16, kind="ExternalInput")
    b2_d = nc.dram_tensor("b2", [DIM], F32, kind="ExternalInput")
    out_d = nc.dram_tensor("out", [n_tok, DIM], F32, kind="ExternalOutput")

    with tile.TileContext(nc) as tc:
        with ExitStack() as ctx:
            consts = ctx.enter_context(tc.tile_pool(name="consts", bufs=1))
            xp = ctx.enter_context(tc.tile_pool(name="xp", bufs=3))
            yp = ctx.enter_context(tc.tile_pool(name="yp", bufs=1))
            ytp = ctx.enter_context(tc.tile_pool(name="ytp", bufs=1))
            qp = ctx.enter_context(tc.tile_pool(name="qp", bufs=2))
            kp = ctx.enter_context(tc.tile_pool(name="kp", bufs=3))
            vp = ctx.enter_context(tc.tile_pool(name="vp", bufs=3))
            ep = ctx.enter_context(tc.tile_pool(name="ep", bufs=2))
            ap_ = ctx.enter_context(tc.tile_pool(name="ap", bufs=2))
            x1p = ctx.enter_context(tc.tile_pool(name="x1p", bufs=5))
            h2p = ctx.enter_context(tc.tile_pool(name="h2p", bufs=1))
            h2tp = ctx.enter_context(tc.tile_pool(name="h2tp", bufs=2))
            gp = ctx.enter_context(tc.tile_pool(name="gp", bufs=1))
            op = ctx.enter_context(tc.tile_pool(name="op", bufs=1))
            tp = ctx.enter_context(tc.tile_pool(name="tp", bufs=2))
            sp = ctx.enter_context(tc.tile_pool(name="sp", bufs=1))
            dp = ctx.enter_context(tc.tile_pool(name="dp", bufs=2, space="DRAM"))
            ps_g = ctx.enter_context(tc.tile_pool(name="ps_g", bufs=2, space="PSUM"))
            ps_s = ctx.enter_context(tc.tile_pool(name="ps_s", bufs=3, space="PSUM"))
            ps_a = ctx.enter_context(tc.tile_pool(name="ps_a", bufs=3, space="PSUM"))

            # ---- constants ----
            qkvw_sb = []
            for c in range(4):
                t = consts.tile([128, 3 * DIM], F16, name=f"qkvw{c}", tag=f"qkvw{c}")
                nc.sync.dma_start(t[:], qkvw_d[c * 128:(c + 1) * 128, :])
                qkvw_sb.append(t)
            projw_sb = []
            for c in range(4):
                t = consts.tile([128, DIM], F16, name=f"projw{c}", tag=f"projw{c}")
                nc.sync.dma_start(t[:], projw_d[c * 128:(c + 1) * 128, :])
                projw_sb.append(t)
            w1_sb = []
            for c in range(4):
                t = consts.tile([128, FF], F16, name=f"w1_{c}", tag=f"w1_{c}")
                nc.sync.dma_start(t[:], w1_d[c * 128:(c + 1) * 128, :])
                w1_sb.append(t)
            w2_sb = []
            for f in range(16):
                t = consts.tile([128, DIM], F16, name=f"w2_{f}", tag=f"w2_{f}")
                nc.sync.dma_start(t[:], w2_d[f * 128:(f + 1) * 128, :])
                w2_sb.append(t)

            projb_bc = consts.tile([128, DIM], F32, name="projb_bc")
            nc.sync.dma_start(projb_bc[:], _bcast_row(projb_d.ap(), 0, DIM))
            b2_bc = consts.tile([128, DIM], F32, name="b2_bc")
            nc.sync.dma_start(b2_bc[:], _bcast_row(b2_d.ap(), 0, DIM))
            qb_sb = consts.tile([128, 4], F32, name="qb_sb")
            nc.sync.dma_start(qb_sb[:], _col_view(qb_d.ap(), 0, 4))
            b1_sb = consts.tile([128, 16], F32, name="b1_sb")
            nc.sync.dma_start(b1_sb[:], _col_view(b1_d.ap(), 0, 16))
            eps_t = consts.tile([128, 1], F32, name="eps_t")
            nc.vector.memset(eps_t[:], EPS)

            # persistent per-group state
            S = {}

            def new_group_state(g):
                S[g] = {}

            # ---------------- stage pieces ----------------
            def prep_load_stats(g):
                st = S[g]
                x_t, mv_t = [], []
                for t in range(4):
                    xt = xp.tile([128, DIM], F32, name=f"x_{g}_{t}", tag=f"x{t}")
                    nc.sync.dma_start(xt[:], x_d[(g * 4 + t) * 128:(g * 4 + t + 1) * 128, :])
                    stats = tp.tile([128, 6], F32, name=f"st_{g}_{t}", tag=f"st{t}")
                    nc.vector.bn_stats(stats[:], xt[:])
                    mv = tp.tile([128, 2], F32, name=f"mv_{g}_{t}", tag=f"mv{t}")
                    nc.vector.bn_aggr(mv[:], stats[:])
                    x_t.append(xt)
                    mv_t.append(mv)
                st["x"] = x_t
                st["mv1"] = mv_t

            def ln2_stats(g):
                st = S[g]
                mv_t = []
                for t in range(4):
                    stats = tp.tile([128, 6], F32, name=f"st2_{g}_{t}", tag=f"st2{t}")
                    nc.vector.bn_stats(stats[:], st["x1"][t][:])
                    mv = tp.tile([128, 2], F32, name=f"mv2_{g}_{t}", tag=f"mv2{t}")
                    nc.vector.bn_aggr(mv[:], stats[:])
                    mv_t.append(mv)
                st["mv2"] = mv_t

            def rsqrt_cluster(g_pre, g_ln2):
                # gather all variances into one tile -> ONE abs_rsqrt instruction
                # (keeps the sqrt act-table from thrashing mid-gelu/exp stream)
                cols = []
                if g_pre is not None:
                    cols += [(S[g_pre], "mv1", t) for t in range(4)]
                if g_ln2 is not None:
                    cols += [(S[g_ln2], "mv2", t) for t in range(4)]
                if not cols:
                    return
                n = len(cols)
                vars_t = tp.tile([128, 8], F32, name=f"vars_{g_pre}_{g_ln2}", tag="vars")
                for idx, (st, key, t) in enumerate(cols):
                    nc.vector.tensor_copy(vars_t[:, idx:idx + 1], st[key][t][:, 1:2])
                rs_all = tp.tile([128, 8], F32, name=f"rs_{g_pre}_{g_ln2}", tag="rsall")
                nc.scalar.activation(rs_all[:, 0:n], vars_t[:, 0:n],
                                     AF.Abs_reciprocal_sqrt, bias=eps_t[:])
                for idx, (st, key, t) in enumerate(cols):
                    st.setdefault("rs1" if key == "mv1" else "rs2", {})[t] = \
                        (rs_all, idx)

            def y_and_transpose(g):
                st = S[g]
                y_t = []
                for t in range(4):
                    yt_ = yp.tile([128, DIM], F16, name=f"y_{g}_{t}", tag=f"y{t}")
                    rs_all, idx = st["rs1"][t]
                    nc.vector.tensor_scalar(yt_[:], st["x"][t][:], st["mv1"][t][:, 0:1],
                                            rs_all[:, idx:idx + 1],
                                            op0=AL.subtract, op1=AL.mult)
                    y_t.append(yt_)
                yT = []
                for c in range(4):
                    t_ = ytp.tile([128, GRP], F16, name=f"yT_{g}_{c}", tag=f"yT{c}")
                    for t in range(4):
                        nc.sync.dma_start_transpose(
                            t_[:, t * 128:(t + 1) * 128],
                            y_t[t][:, c * 128:(c + 1) * 128])
                    yT.append(t_)
                st["yT"] = yT

            def h2_and_transpose(g):
                st = S[g]
                h2_t = []
                for t in range(4):
                    h2 = h2p.tile([128, DIM], F16, name=f"h2_{g}_{t}", tag=f"h2{t}")
                    rs_all, idx = st["rs2"][t]
                    nc.vector.tensor_scalar(h2[:], st["x1"][t][:], st["mv2"][t][:, 0:1],
                                            rs_all[:, idx:idx + 1],
                                            op0=AL.subtract, op1=AL.mult)
                    h2_t.append(h2)
                h2T = []
                for c in range(4):
                    t_ = h2tp.tile([128, GRP], F16, name=f"h2T_{g}_{c}", tag=f"h2T{c}")
                    for t in range(4):
                        nc.sync.dma_start_transpose(
                            t_[:, t * 128:(t + 1) * 128],
                            h2_t[t][:, c * 128:(c + 1) * 128])
                    h2T.append(t_)
                st["h2T"] = h2T

            def mlp1_and_gelu(g):
                st = S[g]
                gel = []
                for f in range(16):
                    P = ps_g.tile([128, GRP], F32, name=f"Pm1_{g}_{f}", tag="gemm")
                    for c in range(4):
                        nc.tensor.matmul(P[:], w1_sb[c][:, f * 128:(f + 1) * 128],
                                         st["h2T"][c][:], start=(c == 0), stop=(c == 3))
                    gl = gp.tile([128, GRP], F16, name=f"gel_{g}_{f}", tag=f"gel{f}")
                    nc.scalar.activation(gl[:], P[:], AF.Gelu, bias=b1_sb[:, f:f + 1])
                    gel.append(gl)
                st["gel"] = gel

            def mlp2_tile(g, t):
                st = S[g]
                P = ps_g.tile([128, DIM], F32, name=f"Pm2_{g}_{t}", tag="gemm")
                for f in range(16):
                    nc.tensor.matmul(P[:], st["gel"][f][:, t * 128:(t + 1) * 128],
                                     w2_sb[f][:], start=(f == 0), stop=(f == 15))
                tt = tp.tile([128, DIM], F32, name=f"ob_{g}_{t}", tag="ob")
                nc.vector.tensor_tensor(tt[:], P[:], b2_bc[:], op=AL.add)
                ot = op.tile([128, DIM], F32, name=f"o_{g}_{t}", tag=f"o{t}")
                nc.vector.tensor_tensor(ot[:], tt[:], st["x1"][t][:], op=AL.add)
                st.setdefault("out", []).append(ot)

            def store_out(g):
                st = S[g]
                for t in range(4):
                    nc.sync.dma_start(out_d[(g * 4 + t) * 128:(g * 4 + t + 1) * 128, :],
                                      st["out"][t][:])

            def qk_chunk(g, m):
                """QK gemm output chunk m (0-3 q, 4-7 k), feature-major."""
                st = S[g]
                P = ps_g.tile([128, GRP], F32, name=f"Pqk_{g}_{m}", tag="gemm")
                for c in range(4):
                    nc.tensor.matmul(P[:], qkvw_sb[c][:, m * 128:(m + 1) * 128],
                                     st["yT"][c][:], start=(c == 0), stop=(c == 3))
                if m < 4:
                    sb = qp.tile([128, GRP], F16, name=f"q_{g}_{m}", tag=f"q{m}")
                    nc.vector.tensor_scalar(sb[:], P[:], qb_sb[:, m:m + 1], None,
                                            op0=AL.add)
                    st.setdefault("q", {})[m] = sb
                else:
                    sb = kp.tile([128, GRP], F16, name=f"k_{g}_{m-4}", tag=f"k{m-4}")
                    nc.vector.tensor_copy(sb[:], P[:])
                    st.setdefault("k", {})[m - 4] = sb

            def v_tile(g, t):
                st = S[g]
                P = ps_g.tile([128, DIM], F32, name=f"Pv_{g}_{t}", tag="gemm")
                for c in range(4):
                    nc.tensor.matmul(P[:], st["yT"][c][:, t * 128:(t + 1) * 128],
                                     qkvw_sb[c][:, 2 * DIM:3 * DIM],
                                     start=(c == 0), stop=(c == 3))
                vt = vp.tile([128, HEADS, HD + 1], F16, name=f"v_{g}_{t}", tag=f"v{t}")
                nc.vector.memset(vt[:, :, HD:HD + 1], 1.0)
                nc.vector.tensor_copy(vt[:, :, 0:HD],
                                      P[:].rearrange("p (h d) -> p h d", h=HEADS))
                st.setdefault("v", {})[t] = vt

            def sims_head(g, h):
                """Keys-stationary sims + exps for head h of group g."""
                st = S[g]
                c, half = h // 2, (h % 2) * 64
                E = {}
                for (j, qlo, qhi) in _sim_spans(g, n_blocks):
                    span = qhi - qlo
                    gj, s = divmod(j, 4)
                    P = ps_s.tile([128, 384], F32, name=f"Ps_{g}_{h}_{j}", tag="sim")
                    nc.tensor.matmul(
                        P[:, 0:span],
                        S[gj]["k"][c][half:half + 64, s * 128:(s + 1) * 128],
                        st["q"][c][half:half + 64, qlo:qhi],
                        start=True, stop=True)
                    Et = ep.tile([128, 384], F16, name=f"E_{g}_{h}_{j}", tag=f"E{j%6}")
                    nc.scalar.activation(Et[:, 0:span], P[:, 0:span], AF.Exp,
                                         scale=float(HD) ** -0.5)
                    E[j] = (Et, qlo)
                st.setdefault("E", {})[h] = E

            def av_head(g, h):
                st = S[g]
                E = st["E"][h]
                P_av = ps_a.tile([65, GRP], F32, name=f"Pav_{g}_{h}", tag="av")
                for (j, a, b, start) in _av_plan(g, n_blocks):
                    gj, s = divmod(j, 4)
                    Et, qlo = E[j]
                    nc.tensor.matmul(
                        P_av[:, a:b],
                        S[gj]["v"][s][:, h, :],
                        Et[:, a - qlo:b - qlo],
                        start=start, stop=True, skip_group_check=True)
                del st["E"][h]
                # drain PSUM immediately: unnormalized attn rows + sums row;
                # DMA-free normalize chain (xbar transposes hog the DMA path)
                p, half = h // 2, (h % 2) * 64
                if h % 2 == 0:
                    st.setdefault("attn", {})[p] = ap_.tile(
                        [128, GRP], F16, name=f"attn_{g}_{p}", tag=f"attn{p}")
                at = st["attn"][p]
                with nc.allow_low_precision(reason="unnormalized attn fp16"):
                    nc.vector.tensor_copy(at[half:half + 64, :], P_av[0:64, :])
                ssh = sp.tile([1, GRP], F32, name=f"sr_{g}_{h}", tag="sr")
                nc.scalar.activation(ssh[:], P_av[64:65, :], AF.Copy)
                r_h = sp.tile([1, GRP], F32, name=f"rr_{g}_{h}", tag=f"rr{h % 2}")
                nc.vector.reciprocal_approx_fast(r_h[:], ssh[:])
                rbc = sp.tile([128, GRP], F32, name=f"rb_{g}_{h}", tag="rb")
                nc.gpsimd.partition_broadcast(rbc[:], r_h[:], channels=128)
                with nc.allow_low_precision(reason="attn normalize fp16"):
                    nc.vector.tensor_tensor(at[half:half + 64, :],
                                            at[half:half + 64, :],
                                            rbc[half:half + 64, :], op=AL.mult)

            def norm_finish(g):
                pass

            def proj_x1(g):
                st = S[g]
                x1_t = []
                for t in range(4):
                    P = ps_g.tile([128, DIM], F32, name=f"Ppr_{g}_{t}", tag="gemm")
                    for c in range(4):
                        nc.tensor.matmul(P[:], st["attn"][c][:, t * 128:(t + 1) * 128],
                                         projw_sb[c][:], start=(c == 0), stop=(c == 3))
                    tt = tp.tile([128, DIM], F32, name=f"pb_{g}_{t}", tag="pb")
                    nc.vector.tensor_tensor(tt[:], P[:], projb_bc[:], op=AL.add)
                    x1 = x1p.tile([128, DIM], F16, name=f"x1_{g}_{t}", tag=f"x1{t}")
                    with nc.allow_low_precision(reason="x1 residual fp16"):
                        nc.vector.tensor_tensor(x1[:], tt[:], st["x"][t][:], op=AL.add)
                    x1_t.append(x1)
                st["x1"] = x1_t

            # ---------------- main pipeline ----------------
            QK_ORDER = [4, 5, 6, 7, 0, 1, 2, 3]
            total_iters = n_groups + 7
            for i in range(total_iters):
                g_pre = i if i < n_groups else None
                g_qkv = i - 1 if 0 <= i - 1 < n_groups else None
                g_att = i - 2 if 0 <= i - 2 < n_groups else None
                g_ln2 = i - 3 if 0 <= i - 3 < n_groups else None
                g_mlp1 = i - 5 if 0 <= i - 5 < n_groups else None
                g_mlp2 = i - 6 if 0 <= i - 6 < n_groups else None

                if g_pre is not None:
                    new_group_state(g_pre)
                    prep_load_stats(g_pre)
                if g_ln2 is not None:
                    ln2_stats(g_ln2)

                # attention + qkv + MLP2 slots
                for h in range(8):
                    if g_qkv is not None:
                        qk_chunk(g_qkv, QK_ORDER[h])
                        if h < 4:
                            v_tile(g_qkv, h)
                    if g_att is not None:
                        sims_head(g_att, h)
                        if h >= 1:
                            av_head(g_att, h - 1)
                    if g_mlp2 is not None and h % 2 == 1:
                        mlp2_tile(g_mlp2, h // 2)
                if g_att is not None:
                    av_head(g_att, 7)
                    norm_finish(g_att)

                # sqrt cluster (one table load), then y/h2 + transposes
                rsqrt_cluster(g_pre, g_ln2)
                if g_pre is not None:
                    y_and_transpose(g_pre)
                if g_ln2 is not None:
                    h2_and_transpose(g_ln2)

                # MLP1 + gelus at iteration end: gelu drain keeps pace with PE
                if g_mlp1 is not None:
                    mlp1_and_gelu(g_mlp1)
                if g_mlp2 is not None:
                    store_out(g_mlp2)

                if g_att is not None:
                    proj_x1(g_att)

                # drop references no longer needed to let pools recycle
                g_done = i - 7
                if 0 <= g_done < n_groups:
                    S.pop(g_done, None)

    nc.compile()
    return nc


_cache = {}


def _get_nc(n_tok):
    if n_tok not in _cache:
        _cache[n_tok] = build(n_tok)
    return _cache[n_tok]


def _prep_in_maps(inputs):
    return _prep(**inputs)


def _prep(x, t_emb, ln1_g, ln1_b, qkv_w, qkv_b, proj_w, proj_b,
          ln2_g, ln2_b, mlp_w1, mlp_b1, mlp_w2, mlp_b2, time_w, time_b):
    x = np.asarray(x, dtype=np.float32)
    t_emb = np.asarray(t_emb, np.float32)
    qkv_w = np.asarray(qkv_w, np.float32)
    qkv_b = np.asarray(qkv_b, np.float32)
    proj_w = np.asarray(proj_w, np.float32)
    proj_b = np.asarray(proj_b, np.float32)

    s = t_emb / (1.0 + np.exp(-t_emb))           # silu
    ss = s @ np.asarray(time_w, np.float32) + np.asarray(time_b, np.float32)
    scale, shift = ss[:, :DIM], ss[:, DIM:]
    g1 = np.asarray(ln1_g, np.float32)
    be1 = np.asarray(ln1_b, np.float32)
    arow = g1[None, :] * (1.0 + scale)                      # [B, 512]
    crow = be1[None, :] * (1.0 + scale) + shift             # [B, 512]

    # fold modulation into qkv weights/bias (per batch):
    #   y_mod = xh*arow + crow ;  qkv = y_mod @ W + b = xh @ (arow[:,None]*W) + (b + crow@W)
    nb = x.shape[0]
    qkvw_b16 = np.empty((nb, DIM, 3 * DIM), np.float16)
    qb_full = np.empty((nb, 3 * DIM), np.float32)
    for b in range(nb):
        qkvw_b16[b] = (qkv_w * arow[b][:, None]).astype(np.float16)
        qb_full[b] = qkv_b + crow[b] @ qkv_w
    # k bias dropped (softmax-invariant); v bias folded into proj bias
    pb = proj_b[None, :] + qb_full[:, 2 * DIM:] @ proj_w     # [B, 512]

    # fold ln2 gamma/beta into mlp_w1/b1
    g2 = np.asarray(ln2_g, np.float32)
    be2 = np.asarray(ln2_b, np.float32)
    w1f = (np.asarray(mlp_w1, np.float32) * g2[:, None]).astype(np.float16)
    b1f = be2 @ np.asarray(mlp_w1, np.float32) + np.asarray(mlp_b1, np.float32)

    projw16 = proj_w.astype(np.float16)
    w216 = np.asarray(mlp_w2, np.float32).astype(np.float16)
    b2 = np.asarray(mlp_b2, np.float32)

    in_maps = []
    for b in range(nb):
        in_maps.append({
            "x": np.ascontiguousarray(x[b]),
            "qkvw": np.ascontiguousarray(qkvw_b16[b]),
            "qb": np.ascontiguousarray(qb_full[b, :DIM]),
            "projw": projw16,
            "projb": np.ascontiguousarray(pb[b]),
            "w1": w1f, "b1": b1f, "w2": w216, "b2": b2,
        })
    return in_maps


def kernel(**inputs):
    in_maps = _prep_in_maps(inputs)
    n_tok = in_maps[0]["x"].shape[0]
    nc = _get_nc(n_tok)
    nb = len(in_maps)
    res = bass_utils.run_bass_kernel_spmd(nc, in_maps, core_ids=list(range(nb)))
    out = np.stack([res.results[b]["out"] for b in range(nb)], axis=0)
    return out


# revision 28
# speedup vs baseline: 1.2502x; 1.2502x over previous
"""Trainium2 Bass kernel for a local-attention transformer block (v2).

Per batch element (one NeuronCore each, 8 cores):
  y   = LN(x)  (gamma/beta and AdaLN scale/shift folded into qkv weights host-side)
  q,k,v = y @ qkv_w' + b'   (k bias dropped: softmax-invariant; v bias folded into proj bias)
  attn: each 128-token window attends [prev|cur|next]; keys-stationary sims,
        ones-column-in-v denominator trick, DVE-fused normalize from PSUM.
  x1  = x + attn @ proj_w + pb'   (pb' = proj_b + v_bias @ proj_w, per batch)
  out = x1 + gelu(LN2(x1) @ w1' + b1') @ w2 + b2   (ln2 g/b folded into w1/b1)

6-deep software pipeline per 512-token group (iteration i):
  prep(i) | qkv(i-1) | attn+proj(i-2) | ln2+h2T(i-3) | (h2T transit) | mlp(i-5)
PE order per iteration: MLP1, [slots: QK/V + sims + AV], MLP2, proj —
keeps the PE continuously busy (p-state) while ACT grinds exps/gelus.
ACT does exactly 3 table loads per iteration (gelu, exp, abs_rsqrt).
"""

import numpy as np
from contextlib import ExitStack

import concourse.bass as bass
import concourse.tile as tile
from concourse import bacc, mybir
from concourse import bass_utils

F32 = mybir.dt.float32
F16 = mybir.dt.float16
AF = mybir.ActivationFunctionType
AL = mybir.AluOpType

DIM = 512
HEADS = 8
HD = 64
FF = 2048
WIN = 128
NTOK = 8192
EPS = 1e-5
GRP = 512
TS = 128


def _bcast_row(dram_ap, offset, n):
    return bass.AP(tensor=dram_ap.tensor, offset=offset, ap=[[0, 128], [1, n]])


def _col_view(dram_ap, offset, ncol):
    return bass.AP(tensor=dram_ap.tensor, offset=offset, ap=[[1, 128], [128, ncol]])


def _sim_spans(g, n_blocks):
    """[(j, qlo, qhi)] for group g: key block j vs in-group query columns."""
    out = []
    for j in range(4 * g - 1, 4 * g + 5):
        if 0 <= j < n_blocks:
            qlo = (max(j - 1, 4 * g) - 4 * g) * WIN
            qhi = (min(j + 1, 4 * g + 3, n_blocks - 1) - 4 * g) * WIN + WIN
            out.append((j, qlo, qhi))
    return out


def _av_plan(g, n_blocks):
    """[(j, a, b, start)] accumulation plan into P_av[65, 512]."""
    plan = [(4 * g, 0, 256, True), (4 * g + 1, 0, 256, False)]
    if 4 * g - 1 >= 0:
        plan.append((4 * g - 1, 0, 128, False))
    plan.append((4 * g + 2, 128, 256, False))
    plan += [(4 * g + 2, 256, 512, True), (4 * g + 3, 256, 512, False),
             (4 * g + 1, 256, 384, False)]
    if 4 * g + 4 < n_blocks:
        plan.append((4 * g + 4, 384, 512, False))
    return plan


def build(n_tok=NTOK):
    n_groups = n_tok // GRP
    n_blocks = n_tok // WIN
    nc = bacc.Bacc("TRN2", target_bir_lowering=False, debug=False)

    x_d = nc.dram_tensor("x", [n_tok, DIM], F32, kind="ExternalInput")
    qkvw_d = nc.dram_tensor("qkvw", [DIM, 3 * DIM], F16, kind="ExternalInput")
    qb_d = nc.dram_tensor("qb", [DIM], F32, kind="ExternalInput")
    projw_d = nc.dram_tensor("projw", [DIM, DIM], F16, kind="ExternalInput")
    projb_d = nc.dram_tensor("projb", [DIM], F32, kind="ExternalInput")
    w1_d = nc.dram_tensor("w1", [DIM, FF], F16, kind="ExternalInput")
    b1_d = nc.dram_tensor("b1", [FF], F32, kind="ExternalInput")
    w2_d = nc.dram_tensor("w2", [FF, DIM], F16, kind="ExternalInput")
    b2_d = nc.dram_tensor("b2", [DIM], F32, kind="ExternalInput")
    out_d = nc.dram_tensor("out", [n_tok, DIM], F32, kind="ExternalOutput")

    with tile.TileContext(nc) as tc:
        with ExitStack() as ctx:
            consts = ctx.enter_context(tc.tile_pool(name="consts", bufs=1))
            xp = ctx.enter_context(tc.tile_pool(name="xp", bufs=3))
            yp = ctx.enter_context(tc.tile_pool(name="yp", bufs=1))
            ytp = ctx.enter_context(tc.tile_pool(name="ytp", bufs=1))
            qp = ctx.enter_context(tc.tile_pool(name="qp", bufs=2))
            kp = ctx.enter_context(tc.tile_pool(name="kp", bufs=3))
            vp = ctx.enter_context(tc.tile_pool(name="vp", bufs=3))
            ep = ctx.enter_context(tc.tile_pool(name="ep", bufs=2))
            ap_ = ctx.enter_context(tc.tile_pool(name="ap", bufs=2))
            x1p = ctx.enter_context(tc.tile_pool(name="x1p", bufs=4))
            h2p = ctx.enter_context(tc.tile_pool(name="h2p", bufs=1))
            h2tp = ctx.enter_context(tc.tile_pool(name="h2tp", bufs=2))
            gp = ctx.enter_context(tc.tile_pool(name="gp", bufs=1))
            op = ctx.enter_context(tc.tile_pool(name="op", bufs=1))
            tp = ctx.enter_context(tc.tile_pool(name="tp", bufs=2))
            sp = ctx.enter_context(tc.tile_pool(name="sp", bufs=1))
            dp = ctx.enter_context(tc.tile_pool(name="dp", bufs=2, space="DRAM"))
            ps_g = ctx.enter_context(tc.tile_pool(name="ps_g", bufs=3, space="PSUM"))
            ps_m = ctx.enter_context(tc.tile_pool(name="ps_m", bufs=2, space="PSUM"))
            ps_s = ctx.enter_context(tc.tile_pool(name="ps_s", bufs=3, space="PSUM"))

            # ---- constants ----
            qkvw_sb = []
            for c in range(4):
                t = consts.tile([128, 3 * DIM], F16, name=f"qkvw{c}", tag=f"qkvw{c}")
                nc.sync.dma_start(t[:], qkvw_d[c * 128:(c + 1) * 128, :])
                qkvw_sb.append(t)
            projw_sb = []
            for c in range(4):
                t = consts.tile([128, DIM], F16, name=f"projw{c}", tag=f"projw{c}")
                nc.sync.dma_start(t[:], projw_d[c * 128:(c + 1) * 128, :])
                projw_sb.append(t)
            w1_sb = []
            for c in range(4):
                t = consts.tile([128, FF], F16, name=f"w1_{c}", tag=f"w1_{c}")
                nc.sync.dma_start(t[:], w1_d[c * 128:(c + 1) * 128, :])
                w1_sb.append(t)
            w2_sb = []
            for f in range(16):
                t = consts.tile([128, DIM], F16, name=f"w2_{f}", tag=f"w2_{f}")
                nc.sync.dma_start(t[:], w2_d[f * 128:(f + 1) * 128, :])
                w2_sb.append(t)

            projb_bc = consts.tile([128, DIM], F32, name="projb_bc")
            nc.sync.dma_start(projb_bc[:], _bcast_row(projb_d.ap(), 0, DIM))
            b2_bc = consts.tile([128, DIM], F32, name="b2_bc")
            nc.sync.dma_start(b2_bc[:], _bcast_row(b2_d.ap(), 0, DIM))
            qb_sb = consts.tile([128, 4], F32, name="qb_sb")
            nc.sync.dma_start(qb_sb[:], _col_view(qb_d.ap(), 0, 4))
            b1_sb = consts.tile([128, 16], F32, name="b1_sb")
            nc.sync.dma_start(b1_sb[:], _col_view(b1_d.ap(), 0, 16))
            eps_t = consts.tile([128, 1], F32, name="eps_t")
            nc.vector.memset(eps_t[:], EPS)

            # persistent per-group state
            S = {}

            def new_group_state(g):
                S[g] = {}

            # ---------------- stage pieces ----------------
            def prep_load_stats(g):
                st = S[g]
                x_t, mv_t = [], []
                for t in range(4):
                    xt = xp.tile([128, DIM], F32, name=f"x_{g}_{t}", tag=f"x{t}")
                    nc.sync.dma_start(xt[:], x_d[(g * 4 + t) * 128:(g * 4 + t + 1) * 128, :])
                    stats = tp.tile([128, 6], F32, name=f"st_{g}_{t}", tag=f"st{t}")
                    nc.vector.bn_stats(stats[:], xt[:])
                    mv = tp.tile([128, 2], F32, name=f"mv_{g}_{t}", tag=f"mv{t}")
                    nc.vector.bn_aggr(mv[:], stats[:])
                    x_t.append(xt)
                    mv_t.append(mv)
                st["x"] = x_t
                st["mv1"] = mv_t

            def ln2_stats(g):
                st = S[g]
                mv_t = []
                for t in range(4):
                    stats = tp.tile([128, 6], F32, name=f"st2_{g}_{t}", tag=f"st2{t}")
                    nc.vector.bn_stats(stats[:], st["x1"][t][:])
                    mv = tp.tile([128, 2], F32, name=f"mv2_{g}_{t}", tag=f"mv2{t}")
                    nc.vector.bn_aggr(mv[:], stats[:])
                    mv_t.append(mv)
                st["mv2"] = mv_t

            def rsqrt_cluster(g_pre, g_ln2):
                # gather all variances into one tile -> ONE abs_rsqrt instruction
                # (keeps the sqrt act-table from thrashing mid-gelu/exp stream)
                cols = []
                if g_pre is not None:
                    cols += [(S[g_pre], "mv1", t) for t in range(4)]
                if g_ln2 is not None:
                    cols += [(S[g_ln2], "mv2", t) for t in range(4)]
                if not cols:
                    return
                n = len(cols)
                vars_t = tp.tile([128, 8], F32, name=f"vars_{g_pre}_{g_ln2}", tag="vars")
                for idx, (st, key, t) in enumerate(cols):
                    nc.vector.tensor_copy(vars_t[:, idx:idx + 1], st[key][t][:, 1:2])
                rs_all = tp.tile([128, 8], F32, name=f"rs_{g_pre}_{g_ln2}", tag="rsall")
                nc.scalar.activation(rs_all[:, 0:n], vars_t[:, 0:n],
                                     AF.Abs_reciprocal_sqrt, bias=eps_t[:])
                for idx, (st, key, t) in enumerate(cols):
                    st.setdefault("rs1" if key == "mv1" else "rs2", {})[t] = \
                        (rs_all, idx)

            def y_and_transpose(g):
                st = S[g]
                y_t = []
                for t in range(4):
                    yt_ = yp.tile([128, DIM], F16, name=f"y_{g}_{t}", tag=f"y{t}")
                    rs_all, idx = st["rs1"][t]
                    nc.vector.tensor_scalar(yt_[:], st["x"][t][:], st["mv1"][t][:, 0:1],
                                            rs_all[:, idx:idx + 1],
                                            op0=AL.subtract, op1=AL.mult)
                    y_t.append(yt_)
                yT = []
                for c in range(4):
                    t_ = ytp.tile([128, GRP], F16, name=f"yT_{g}_{c}", tag=f"yT{c}")
                    for t in range(4):
                        nc.sync.dma_start_transpose(
                            t_[:, t * 128:(t + 1) * 128],
                            y_t[t][:, c * 128:(c + 1) * 128])
                    yT.append(t_)
                st["yT"] = yT

            def h2_and_transpose(g):
                st = S[g]
                h2_t = []
                for t in range(4):
                    h2 = h2p.tile([128, DIM], F16, name=f"h2_{g}_{t}", tag=f"h2{t}")
                    rs_all, idx = st["rs2"][t]
                    nc.vector.tensor_scalar(h2[:], st["x1"][t][:], st["mv2"][t][:, 0:1],
                                            rs_all[:, idx:idx + 1],
                                            op0=AL.subtract, op1=AL.mult)
                    h2_t.append(h2)
                h2T = []
                for c in range(4):
                    t_ = h2tp.tile([128, GRP], F16, name=f"h2T_{g}_{c}", tag=f"h2T{c}")
                    for t in range(4):
                        nc.sync.dma_start_transpose(
                            t_[:, t * 128:(t + 1) * 128],
                            h2_t[t][:, c * 128:(c + 1) * 128])
                    h2T.append(t_)
                st["h2T"] = h2T

            def mlp1_and_gelu(g):
                st = S[g]
                gel = []
                for f in range(16):
                    P = ps_m.tile([128, GRP], F32, name=f"Pm1_{g}_{f}", tag="m1")
                    for c in range(4):
                        nc.tensor.matmul(P[:], w1_sb[c][:, f * 128:(f + 1) * 128],
                                         st["h2T"][c][:], start=(c == 0), stop=(c == 3))
                    gl = gp.tile([128, GRP], F16, name=f"gel_{g}_{f}", tag=f"gel{f}")
                    nc.scalar.activation(gl[:], P[:], AF.Gelu, bias=b1_sb[:, f:f + 1])
                    gel.append(gl)
                st["gel"] = gel

            def mlp2_tile(g, t):
                st = S[g]
                P = ps_g.tile([128, DIM], F32, name=f"Pm2_{g}_{t}", tag="gemm")
                for f in range(16):
                    nc.tensor.matmul(P[:], st["gel"][f][:, t * 128:(t + 1) * 128],
                                     w2_sb[f][:], start=(f == 0), stop=(f == 15))
                tt = tp.tile([128, DIM], F32, name=f"ob_{g}_{t}", tag="ob")
                nc.vector.tensor_tensor(tt[:], P[:], b2_bc[:], op=AL.add)
                ot = op.tile([128, DIM], F32, name=f"o_{g}_{t}", tag=f"o{t}")
                nc.vector.tensor_tensor(ot[:], tt[:], st["x1"][t][:], op=AL.add)
                st.setdefault("out", []).append(ot)

            def store_out(g):
                st = S[g]
                for t in range(4):
                    nc.sync.dma_start(out_d[(g * 4 + t) * 128:(g * 4 + t + 1) * 128, :],
                                      st["out"][t][:])

            def qk_chunk(g, m):
                """QK gemm output chunk m (0-3 q, 4-7 k), feature-major."""
                st = S[g]
                P = ps_g.tile([128, GRP], F32, name=f"Pqk_{g}_{m}", tag="gemm")
                for c in range(4):
                    nc.tensor.matmul(P[:], qkvw_sb[c][:, m * 128:(m + 1) * 128],
                                     st["yT"][c][:], start=(c == 0), stop=(c == 3))
                if m < 4:
                    sb = qp.tile([128, GRP], F16, name=f"q_{g}_{m}", tag=f"q{m}")
                    nc.vector.tensor_scalar(sb[:], P[:], qb_sb[:, m:m + 1], None,
                                            op0=AL.add)
                    st.setdefault("q", {})[m] = sb
                else:
                    sb = kp.tile([128, GRP], F16, name=f"k_{g}_{m-4}", tag=f"k{m-4}")
                    nc.vector.tensor_copy(sb[:], P[:])
                    st.setdefault("k", {})[m - 4] = sb

            def v_tile(g, t):
                st = S[g]
                P = ps_g.tile([128, DIM], F32, name=f"Pv_{g}_{t}", tag="gemm")
                for c in range(4):
                    nc.tensor.matmul(P[:], st["yT"][c][:, t * 128:(t + 1) * 128],
                                     qkvw_sb[c][:, 2 * DIM:3 * DIM],
                                     start=(c == 0), stop=(c == 3))
                vt = vp.tile([128, HEADS, HD + 1], F16, name=f"v_{g}_{t}", tag=f"v{t}")
                nc.vector.memset(vt[:, :, HD:HD + 1], 1.0)
                nc.vector.tensor_copy(vt[:, :, 0:HD],
                                      P[:].rearrange("p (h d) -> p h d", h=HEADS))
                st.setdefault("v", {})[t] = vt

            def sims_head(g, h):
                """Keys-stationary sims + exps for head h of group g."""
                st = S[g]
                c, half = h // 2, (h % 2) * 64
                E = {}
                for (j, qlo, qhi) in _sim_spans(g, n_blocks):
                    span = qhi - qlo
                    gj, s = divmod(j, 4)
                    P = ps_s.tile([128, 384], F32, name=f"Ps_{g}_{h}_{j}", tag="sim")
                    nc.tensor.matmul(
                        P[:, 0:span],
                        S[gj]["k"][c][half:half + 64, s * 128:(s + 1) * 128],
                        st["q"][c][half:half + 64, qlo:qhi],
                        start=True, stop=True)
                    Et = ep.tile([128, 384], F16, name=f"E_{g}_{h}_{j}", tag=f"E{j%6}")
                    nc.scalar.activation(Et[:, 0:span], P[:, 0:span], AF.Exp,
                                         scale=float(HD) ** -0.5)
                    E[j] = (Et, qlo)
                st.setdefault("E", {})[h] = E

            def av_head(g, h):
                st = S[g]
                E = st["E"][h]
                P_av = ps_g.tile([65, GRP], F32, name=f"Pav_{g}_{h}", tag="gemm")
                for (j, a, b, start) in _av_plan(g, n_blocks):
                    gj, s = divmod(j, 4)
                    Et, qlo = E[j]
                    nc.tensor.matmul(
                        P_av[:, a:b],
                        S[gj]["v"][s][:, h, :],
                        Et[:, a - qlo:b - qlo],
                        start=start, stop=True, skip_group_check=True)
                del st["E"][h]
                # drain PSUM immediately: unnormalized attn rows + sums row;
                # DMA-free normalize chain (xbar transposes hog the DMA path)
                p, half = h // 2, (h % 2) * 64
                if h % 2 == 0:
                    st.setdefault("attn", {})[p] = ap_.tile(
                        [128, GRP], F16, name=f"attn_{g}_{p}", tag=f"attn{p}")
                at = st["attn"][p]
                with nc.allow_low_precision(reason="unnormalized attn fp16"):
                    nc.vector.tensor_copy(at[half:half + 64, :], P_av[0:64, :])
                ssh = sp.tile([1, GRP], F32, name=f"sr_{g}_{h}", tag="sr")
                nc.scalar.activation(ssh[:], P_av[64:65, :], AF.Copy)
                r_h = sp.tile([1, GRP], F32, name=f"rr_{g}_{h}", tag=f"rr{h % 2}")
                nc.vector.reciprocal_approx_fast(r_h[:], ssh[:])
                rbc = sp.tile([128, GRP], F32, name=f"rb_{g}_{h}", tag="rb")
                nc.gpsimd.partition_broadcast(rbc[:], r_h[:], channels=128)
                with nc.allow_low_precision(reason="attn normalize fp16"):
                    nc.vector.tensor_tensor(at[half:half + 64, :],
                                            at[half:half + 64, :],
                                            rbc[half:half + 64, :], op=AL.mult)

            def norm_finish(g):
                pass

            def proj_x1(g):
                st = S[g]
                x1_t = []
                for t in range(4):
                    P = ps_g.tile([128, DIM], F32, name=f"Ppr_{g}_{t}", tag="gemm")
                    for c in range(4):
                        nc.tensor.matmul(P[:], st["attn"][c][:, t * 128:(t + 1) * 128],
                                         projw_sb[c][:], start=(c == 0), stop=(c == 3))
                    tt = tp.tile([128, DIM], F32, name=f"pb_{g}_{t}", tag="pb")
                    nc.vector.tensor_tensor(tt[:], P[:], projb_bc[:], op=AL.add)
                    x1 = x1p.tile([128, DIM], F16, name=f"x1_{g}_{t}", tag=f"x1{t}")
                    with nc.allow_low_precision(reason="x1 residual fp16"):
                        nc.vector.tensor_tensor(x1[:], tt[:], st["x"][t][:], op=AL.add)
                    x1_t.append(x1)
                st["x1"] = x1_t

            # ---------------- main pipeline ----------------
            QK_ORDER = [4, 5, 6, 7, 0, 1, 2, 3]
            total_iters = n_groups + 6
            for i in range(total_iters):
                g_pre = i if i < n_groups else None
                g_qkv = i - 1 if 0 <= i - 1 < n_groups else None
                g_att = i - 2 if 0 <= i - 2 < n_groups else None
                g_ln2 = i - 3 if 0 <= i - 3 < n_groups else None
                g_mlp = i - 5 if 0 <= i - 5 < n_groups else None

                if g_pre is not None:
                    new_group_state(g_pre)
                    prep_load_stats(g_pre)
                if g_ln2 is not None:
                    ln2_stats(g_ln2)

                # MLP1 + gelus first (own PSUM pool; gelu keeps pace with PE)
                if g_mlp is not None:
                    mlp1_and_gelu(g_mlp)

                # attention + qkv + MLP2 slots
                for h in range(8):
                    if g_qkv is not None:
                        qk_chunk(g_qkv, QK_ORDER[h])
                        if h < 4:
                            v_tile(g_qkv, h)
                    if g_att is not None:
                        sims_head(g_att, h)
                        if h >= 1:
                            av_head(g_att, h - 1)
                    if g_mlp is not None and h % 2 == 1:
                        mlp2_tile(g_mlp, h // 2)
                if g_att is not None:
                    av_head(g_att, 7)
                    norm_finish(g_att)

                # sqrt cluster (one table load), then y/h2 + transposes
                rsqrt_cluster(g_pre, g_ln2)
                if g_pre is not None:
                    y_and_transpose(g_pre)
                if g_ln2 is not None:
                    h2_and_transpose(g_ln2)

                if g_mlp is not None:
                    store_out(g_mlp)

                if g_att is not None:
                    proj_x1(g_att)

                # drop references no longer needed to let pools recycle
                g_done = i - 6
                if 0 <= g_done < n_groups:
                    S.pop(g_done, None)

    nc.compile()
    return nc


_cache = {}


def _get_nc(n_tok):
    if n_tok not in _cache:
        _cache[n_tok] = build(n_tok)
    return _cache[n_tok]


def _prep_in_maps(inputs):
    return _prep(**inputs)


def _prep(x, t_emb, ln1_g, ln1_b, qkv_w, qkv_b, proj_w, proj_b,
          ln2_g, ln2_b, mlp_w1, mlp_b1, mlp_w2, mlp_b2, time_w, time_b):
    x = np.asarray(x, dtype=np.float32)
    t_emb = np.asarray(t_emb, np.float32)
    qkv_w = np.asarray(qkv_w, np.float32)
    qkv_b = np.asarray(qkv_b, np.float32)
    proj_w = np.asarray(proj_w, np.float32)
    proj_b = np.asarray(proj_b, np.float32)

    s = t_emb / (1.0 + np.exp(-t_emb))           # silu
    ss = s @ np.asarray(time_w, np.float32) + np.asarray(time_b, np.float32)
    scale, shift = ss[:, :DIM], ss[:, DIM:]
    g1 = np.asarray(ln1_g, np.float32)
    be1 = np.asarray(ln1_b, np.float32)
    arow = g1[None, :] * (1.0 + scale)                      # [B, 512]
    crow = be1[None, :] * (1.0 + scale) + shift             # [B, 512]

    # fold modulation into qkv weights/bias (per batch):
    #   y_mod = xh*arow + crow ;  qkv = y_mod @ W + b = xh @ (arow[:,None]*W) + (b + crow@W)
    nb = x.shape[0]
    qkvw_b16 = np.empty((nb, DIM, 3 * DIM), np.float16)
    qb_full = np.empty((nb, 3 * DIM), np.float32)
    for b in range(nb):
        qkvw_b16[b] = (qkv_w * arow[b][:, None]).astype(np.float16)
        qb_full[b] = qkv_b + crow[b] @ qkv_w
    # k bias dropped (softmax-invariant); v bias folded into proj bias
    pb = proj_b[None, :] + qb_full[:, 2 * DIM:] @ proj_w     # [B, 512]

    # fold ln2 gamma/beta into mlp_w1/b1
    g2 = np.asarray(ln2_g, np.float32)
    be2 = np.asarray(ln2_b, np.float32)
    w1f = (np.asarray(mlp_w1, np.float32) * g2[:, None]).astype(np.float16)
    b1f = be2 @ np.asarray(mlp_w1, np.float32) + np.asarray(mlp_b1, np.float32)

    projw16 = proj_w.astype(np.float16)
    w216 = np.asarray(mlp_w2, np.float32).astype(np.float16)
    b2 = np.asarray(mlp_b2, np.float32)

    in_maps = []
    for b in range(nb):
        in_maps.append({
            "x": np.ascontiguousarray(x[b]),
            "qkvw": np.ascontiguousarray(qkvw_b16[b]),
            "qb": np.ascontiguousarray(qb_full[b, :DIM]),
            "projw": projw16,
            "projb": np.ascontiguousarray(pb[b]),
            "w1": w1f, "b1": b1f, "w2": w216, "b2": b2,
        })
    return in_maps


def kernel(**inputs):
    in_maps = _prep_in_maps(inputs)
    n_tok = in_maps[0]["x"].shape[0]
    nc = _get_nc(n_tok)
    nb = len(in_maps)
    res = bass_utils.run_bass_kernel_spmd(nc, in_maps, core_ids=list(range(nb)))
    out = np.stack([res.results[b]["out"] for b in range(nb)], axis=0)
    return out
